# revision 1
# baseline (speedup 1.0000x reference)
"""Causal self-attention (b=2, t=2048, d=1024, h=16) on 8 trn2 NeuronCores.

Sharding: core c handles batch c//4 and the 4 heads 4*(c%4)..4*(c%4)+3
(data parallel over batch x tensor parallel over heads). Each core
computes x @ w_qkv for its head-slice, causal attention for its heads,
and a partial out-projection  y_heads @ w_out[head_rows]; the host sums
the 4 partial outputs per batch (the tensor-parallel all-reduce).

Per-core kernel layout (everything f32r = fp32 storage, reduced-precision
matmul mode, 1 cyc/row on the PE at N>=256):
  xT [d, t] built via bf16 hi/lo split + DMA-transpose + DVE add
  qT, kT [dh, t] per head-pair (128 partitions = 2 heads x 64)
  v natural [t, dh] with a fused ones column -> PV matmul emits both
  y_unnorm and the softmax denominator; scores are O(5) so exp needs
  no max-subtraction. S is computed transposed ([j, i]) so softmax
  renormalization is a reciprocal + K=1 broadcast matmul.
"""

import numpy as np
import ml_dtypes

import concourse.bacc as bacc
import concourse.mybir as mybir
import concourse.tile as tile
from concourse.bass_utils import run_bass_kernel_spmd

F32 = mybir.dt.float32
F32R = mybir.dt.float32r
BF16 = mybir.dt.bfloat16

T = 2048            # sequence length
D = 1024            # model dim
DH = 64             # head dim
HPC = 4             # heads per core
NCORES = 8
NTT = T // 128      # 16 t-tiles of 128
NDC = D // 128      # 8 d-chunks of 128
NIB = T // 512      # 4 i-blocks of 512
JPB = 512 // 128    # j-chunks per i-block


def _build():
    nc = bacc.Bacc("TRN2", target_bir_lowering=False, debug=False)

    XHI = nc.dram_tensor("XHI", [T, D], BF16, kind="ExternalInput")
    XLO = nc.dram_tensor("XLO", [T, D], BF16, kind="ExternalInput")
    WQ = nc.dram_tensor("WQ", [D, 256], F32, kind="ExternalInput")
    WK = nc.dram_tensor("WK", [D, 256], F32, kind="ExternalInput")
    WV = nc.dram_tensor("WV", [D, 256], F32, kind="ExternalInput")
    WO = nc.dram_tensor("WO", [256, D], F32, kind="ExternalInput")
    TRI = nc.dram_tensor("TRI", [128, 128], F32, kind="ExternalInput")
    ONESC = nc.dram_tensor("ONESC", [128, NTT, HPC, 1], F32, kind="ExternalInput")
    ONES1 = nc.dram_tensor("ONES1", [1, 64], F32, kind="ExternalInput")
    OUT = nc.dram_tensor("OUT", [T, D], F32, kind="ExternalOutput")

    with tile.TileContext(nc) as tc:
        with tc.tile_pool(name="persist", bufs=1) as pp:
            qt = [pp.tile([128, T], F32R, tag=f"qt{p}", name=f"qt{p}") for p in range(2)]
            kt = [pp.tile([128, T], F32R, tag=f"kt{p}", name=f"kt{p}") for p in range(2)]
            vones = pp.tile([128, NTT, HPC, DH + 1], F32R, tag="vones")
            ypair = [pp.tile([128, T], F32R, tag=f"yp{p}", name=f"yp{p}") for p in range(2)]
            tri = pp.tile([128, 128], F32R, tag="tri")
            ones1 = pp.tile([1, 64], F32R, tag="ones1")
            wo_sb = pp.tile([128, 2, D], F32R, tag="wo")

            with tc.tile_pool(name="ldstage", bufs=2) as lds:
                for dst_ap, src_ap in (
                        (tri[:], TRI[:]),
                        (ones1[:], ONES1[:]),
                        (vones[:, :, :, DH:DH + 1], ONESC[:]),
                        (wo_sb[:], WO[:].rearrange("(c p) e -> p c e", p=128)),
                ):
                    st = lds.tile(list(dst_ap.shape), F32, tag="ldst")
                    nc.sync.dma_start(st[:], src_ap)
                    nc.vector.tensor_copy(dst_ap, st[:])

            # ---------------- phase A: xT + projections ----------------
            with tc.tile_pool(name="phA", bufs=1) as pa, \
                 tc.tile_pool(name="phAhl", bufs=2) as pahl, \
                 tc.tile_pool(name="psA", bufs=4, space="PSUM") as psa:
                xt = pa.tile([128, NDC, T], F32R, tag="xt")
                wq_sb = pa.tile([128, NDC, 256], F32R, tag="wq")
                wk_sb = pa.tile([128, NDC, 256], F32R, tag="wk")
                wv_sb = pa.tile([128, NDC, 256], F32R, tag="wv")
                with tc.tile_pool(name="wstage", bufs=2) as ws:
                    for w_dst, w_src in ((wq_sb, WQ), (wk_sb, WK), (wv_sb, WV)):
                        st = ws.tile([128, NDC, 256], F32, tag="wst")
                        nc.sync.dma_start(
                            st[:], w_src[:].rearrange("(c p) n -> p c n", p=128))
                        nc.vector.tensor_copy(w_dst[:], st[:])

                for dc in range(NDC):
                    xthi = pahl.tile([128, T], BF16, tag="xthi")
                    xtlo = pahl.tile([128, T], BF16, tag="xtlo")
                    nc.sync.dma_start(
                        xthi[:], XHI[:, dc * 128:(dc + 1) * 128], transpose=True)
                    nc.sync.dma_start(
                        xtlo[:], XLO[:, dc * 128:(dc + 1) * 128], transpose=True)
                    nc.vector.tensor_add(xt[:, dc, :], xthi[:], xtlo[:])

                # v projection: v[t, dh] for 4 heads, natural layout
                for ti in range(NTT):
                    vp = psa.tile([128, 256], F32, tag="vp")
                    for dc in range(NDC):
                        nc.tensor.matmul(
                            vp[:], xt[:, dc, ti * 128:(ti + 1) * 128],
                            wv_sb[:, dc, :],
                            start=(dc == 0), stop=(dc == NDC - 1))
                    nc.vector.tensor_copy(
                        vones[:, ti, :, 0:DH],
                        vp[:].rearrange("p (h d) -> p h d", h=HPC))

                # q/k projections, transposed layout, head-pairs of 128
                for dst, w_sb in ((qt, wq_sb), (kt, wk_sb)):
                    for pi in range(2):
                        for ib in range(NIB):
                            qp = psa.tile([128, 512], F32, tag="qkp")
                            for dc in range(NDC):
                                nc.tensor.matmul(
                                    qp[:],
                                    w_sb[:, dc, pi * 128:(pi + 1) * 128],
                                    xt[:, dc, ib * 512:(ib + 1) * 512],
                                    start=(dc == 0), stop=(dc == NDC - 1))
                            nc.vector.tensor_copy(
                                dst[pi][:, ib * 512:(ib + 1) * 512], qp[:])

            # ---------------- phase B: causal attention ----------------
            with tc.tile_pool(name="phB", bufs=1) as pb, \
                 tc.tile_pool(name="phBpt", bufs=3) as pbpt, \
                 tc.tile_pool(name="phBn", bufs=2) as pbn, \
                 tc.tile_pool(name="psBst", bufs=2, space="PSUM") as psbst, \
                 tc.tile_pool(name="psBy", bufs=1, space="PSUM") as psby, \
                 tc.tile_pool(name="psBbc", bufs=2, space="PSUM") as psbbc:
                for pi in range(2):
                    for ib in range(NIB):
                        jlast = JPB * ib + JPB - 1
                        ya = psby.tile([65, 512], F32, tag="ya")
                        yb = psby.tile([65, 512], F32, tag="yb")
                        for jc in range(jlast + 1):
                            off = 128 * (jc - JPB * ib) if jc >= JPB * ib else 0
                            n = 512 - off
                            sta = psbst.tile([128, 512], F32, tag="sta")
                            stb = psbst.tile([128, 512], F32, tag="stb")
                            pta = pbpt.tile([128, 512], F32R, tag="pta")
                            ptb = pbpt.tile([128, 512], F32R, tag="ptb")
                            js = slice(jc * 128, (jc + 1) * 128)
                            isl = slice(ib * 512 + off, (ib + 1) * 512)
                            nc.tensor.matmul(
                                sta[:, off:512], kt[pi][0:64, js],
                                qt[pi][0:64, isl], start=True, stop=True)
                            nc.tensor.matmul(
                                stb[:, off:512], kt[pi][64:128, js],
                                qt[pi][64:128, isl], start=True, stop=True,
                                tile_position=(64, 0))
                            nc.scalar.activation(
                                pta[:, off:512], sta[:, off:512],
                                mybir.ActivationFunctionType.Exp, scale=0.125)
                            nc.scalar.activation(
                                ptb[:, off:512], stb[:, off:512],
                                mybir.ActivationFunctionType.Exp, scale=0.125)
                            if jc >= JPB * ib:  # diagonal chunk: mask triangle
                                nc.vector.tensor_mul(
                                    pta[:, off:off + 128],
                                    pta[:, off:off + 128], tri[:])
                                nc.vector.tensor_mul(
                                    ptb[:, off:off + 128],
                                    ptb[:, off:off + 128], tri[:])
                            nc.tensor.matmul(
                                ya[0:65, off:512], vones[:, jc, 2 * pi, :],
                                pta[:, off:512],
                                start=(jc == 0), stop=(jc == jlast))
                            nc.tensor.matmul(
                                yb[0:65, off:512], vones[:, jc, 2 * pi + 1, :],
                                ptb[:, off:512],
                                start=(jc == 0), stop=(jc == jlast))
                        # renormalize: y /= denom (row 64)
                        ibs = slice(ib * 512, (ib + 1) * 512)
                        for head, yps, rows in ((0, ya, slice(0, 64)),
                                                (1, yb, slice(64, 128))):
                            rec = pbn.tile([1, 512], F32R, tag="rec")
                            bc = psbbc.tile([64, 512], F32, tag="bc")
                            bcs = pbn.tile([64, 512], F32R, tag="bcs")
                            with nc.allow_low_precision(
                                    reason="f32r reciprocal of softmax denom"):
                                nc.vector.reciprocal(rec[:], yps[64:65, :])
                            nc.tensor.matmul(
                                bc[:], ones1[:], rec[:], start=True, stop=True)
                            nc.vector.tensor_copy(bcs[:], bc[:])
                            nc.vector.tensor_mul(
                                ypair[pi][rows, ibs], yps[0:64, :], bcs[:])

            # ---------------- phase C: out-projection ----------------
            with tc.tile_pool(name="phC", bufs=2) as pc_, \
                 tc.tile_pool(name="psC", bufs=4, space="PSUM") as psc:
                for ti in range(NTT):
                    ost = pc_.tile([128, D], F32, tag="ost")
                    for eh in range(2):
                        op = psc.tile([128, 512], F32, tag="op")
                        for pi in range(2):
                            nc.tensor.matmul(
                                op[:], ypair[pi][:, ti * 128:(ti + 1) * 128],
                                wo_sb[:, pi, eh * 512:(eh + 1) * 512],
                                start=(pi == 0), stop=(pi == 1))
                        nc.vector.tensor_copy(
                            ost[:, eh * 512:(eh + 1) * 512], op[:])
                    nc.sync.dma_start(OUT[ti * 128:(ti + 1) * 128, :], ost[:])

    nc.compile()
    return nc


_NC = None


def build_in_maps(x, w_qkv, w_out):
    x = np.asarray(x, np.float32)
    w_qkv = np.asarray(w_qkv, np.float32)
    w_out = np.asarray(w_out, np.float32)

    tri = np.triu(np.ones((128, 128), np.float32))          # tri[j,i]=1 iff j<=i
    onesc = np.ones((128, NTT, HPC, 1), np.float32)
    ones1 = np.ones((1, 64), np.float32)

    in_maps = []
    for c in range(NCORES):
        b, g = divmod(c, 4)
        xb = x[b]
        xhi = xb.astype(ml_dtypes.bfloat16)
        xlo = (xb - xhi.astype(np.float32)).astype(ml_dtypes.bfloat16)
        cs = slice(g * 256, (g + 1) * 256)
        in_maps.append({
            "XHI": xhi, "XLO": xlo,
            "WQ": np.ascontiguousarray(w_qkv[:, cs]),
            "WK": np.ascontiguousarray(w_qkv[:, 1024:2048][:, cs]),
            "WV": np.ascontiguousarray(w_qkv[:, 2048:3072][:, cs]),
            "WO": np.ascontiguousarray(w_out[g * 256:(g + 1) * 256, :]),
            "TRI": tri, "ONESC": onesc, "ONES1": ones1,
        })
    return in_maps


def kernel(x, w_qkv, w_out):
    global _NC
    if _NC is None:
        _NC = _build()

    in_maps = build_in_maps(x, w_qkv, w_out)
    res = run_bass_kernel_spmd(_NC, in_maps, core_ids=list(range(NCORES)))
    outs = [res.results[c]["OUT"] for c in range(NCORES)]
    y = np.stack([outs[0] + outs[1] + outs[2] + outs[3],
                  outs[4] + outs[5] + outs[6] + outs[7]], axis=0)
    return y.astype(np.float32)



# revision 11
# speedup vs baseline: 1.3149x; 1.3149x over previous
"""Causal self-attention (b=2, t=2048, d=1024, h=16) on 8 trn2 NeuronCores.

Sharding: core c handles batch c//4 and the 4 heads 4*(c%4)..4*(c%4)+3
(data parallel over batch x tensor parallel over heads). Each core
computes x @ w_qkv for its head-slice, causal attention for its heads,
and a partial out-projection  y_heads @ w_out[head_rows]; the host sums
the 4 partial outputs per batch (the tensor-parallel all-reduce).

Layout/perf notes:
  x is transposed on the HOST (f32) so the kernel does plain contiguous
  DMAs into f32r tiles (no DMA-transpose, no hi/lo bf16 split, no DVE
  merge). Weights are host-swizzled to [128, chunks, n].
  qT, kT [dh, t] f32r per head-pair (128 partitions = 2 heads x 64).
  S^T is computed per (i-block 512, j-chunk 128) into a 2-bank PSUM tile
  holding BOTH heads of the pair; one ACT instr exps both heads into a
  bf16 P tile (bf16 moving operand keeps 1 cyc/row even for the 128-wide
  diagonal chunks). V is bf16 with a fused ones column so the PV matmul
  emits y_unnorm and the softmax denominator together; scores are O(5)
  so exp needs no max-subtraction. Softmax renorm uses rec = exp(-ln D)
  on ACT (both fns in one table set) instead of the slow DVE iterative
  reciprocal, then a K=1 broadcast matmul. Output is stored bf16 and
  summed in f32 on the host.
"""

import numpy as np
import ml_dtypes

import concourse.bacc as bacc
import concourse.mybir as mybir
import concourse.tile as tile
from concourse.bass_utils import run_bass_kernel_spmd

F32 = mybir.dt.float32
F32R = mybir.dt.float32r
BF16 = mybir.dt.bfloat16
AF = mybir.ActivationFunctionType

T = 2048            # sequence length
D = 1024            # model dim
DH = 64             # head dim
HPC = 4             # heads per core
NCORES = 8
NTT = T // 128      # 16 t-tiles of 128
NDC = D // 128      # 8 d-chunks of 128
NIB = T // 512      # 4 i-blocks of 512
JPB = 512 // 128    # j-chunks per i-block
VW = DH + 2         # v row stride: 64 v + 1 ones + 1 pad (4B alignment)


def _build():
    nc = bacc.Bacc("TRN2", target_bir_lowering=False, debug=False)

    XT = nc.dram_tensor("XT", [128, NDC, T], F32R, kind="ExternalInput")
    WQ = nc.dram_tensor("WQ", [128, NDC, 256], F32R, kind="ExternalInput")
    WK = nc.dram_tensor("WK", [128, NDC, 256], F32R, kind="ExternalInput")
    WV = nc.dram_tensor("WV", [128, NDC, 256], F32R, kind="ExternalInput")
    WO = nc.dram_tensor("WO", [128, 2, D], F32R, kind="ExternalInput")
    TRI = nc.dram_tensor("TRI", [128, 128], BF16, kind="ExternalInput")
    ONESC = nc.dram_tensor("ONESC", [128, NTT, HPC, 1], BF16, kind="ExternalInput")
    ONES1 = nc.dram_tensor("ONES1", [1, 64], F32R, kind="ExternalInput")
    OUT = nc.dram_tensor("OUT", [T, D], BF16, kind="ExternalOutput")

    with tile.TileContext(nc) as tc:
        with tc.tile_pool(name="persist", bufs=1) as pp:
            xt = pp.tile([128, NDC, T], F32R, tag="xt")
            wq_sb = pp.tile([128, NDC, 256], F32R, tag="wq")
            wk_sb = pp.tile([128, NDC, 256], F32R, tag="wk")
            wv_sb = pp.tile([128, NDC, 256], F32R, tag="wv")
            wo_sb = pp.tile([128, 2, D], F32R, tag="wo")
            qt = [pp.tile([128, T], F32R, tag=f"qt{p}", name=f"qt{p}")
                  for p in range(2)]
            kt = [pp.tile([128, T], F32R, tag=f"kt{p}", name=f"kt{p}")
                  for p in range(2)]
            vones = pp.tile([128, NTT, HPC, VW], BF16, tag="vones")
            ypair = [pp.tile([128, T], F32R, tag=f"yp{p}", name=f"yp{p}")
                     for p in range(2)]
            tri = pp.tile([128, 128], BF16, tag="tri")
            ones1 = pp.tile([1, 64], F32R, tag="ones1")

            # input DMAs, in desired landing order: interleave x chunks
            # with the weights each phase needs next.
            nc.sync.dma_start(xt[:, 0, :], XT[:, 0, :])
            nc.sync.dma_start(wv_sb[:], WV[:])
            nc.sync.dma_start(xt[:, 1, :], XT[:, 1, :])
            nc.sync.dma_start(wq_sb[:], WQ[:])
            nc.sync.dma_start(xt[:, 2, :], XT[:, 2, :])
            nc.sync.dma_start(wk_sb[:], WK[:])
            for dc in range(3, NDC):
                nc.sync.dma_start(xt[:, dc, :], XT[:, dc, :])
            nc.sync.dma_start(wo_sb[:], WO[:])
            nc.sync.dma_start(tri[:], TRI[:])
            nc.sync.dma_start(vones[:, :, :, DH:DH + 1], ONESC[:])
            nc.sync.dma_start(ones1[:], ONES1[:])

            # ---------------- phase A: q/k/v projections ----------------
            with tc.tile_pool(name="psv", bufs=3, space="PSUM") as psv, \
                 tc.tile_pool(name="psqk", bufs=5, space="PSUM") as psqk:
                def v_proj(ti):
                    # v[t, dh] for 4 heads, natural layout, bf16
                    vp = psv.tile([128, 256], F32, tag="vp")
                    for dc in range(NDC):
                        nc.tensor.matmul(
                            vp[:], xt[:, dc, ti * 128:(ti + 1) * 128],
                            wv_sb[:, dc, :],
                            start=(dc == 0), stop=(dc == NDC - 1))
                    nc.vector.tensor_copy(
                        vones[:, ti, :, 0:DH],
                        vp[:].rearrange("p (h d) -> p h d", h=HPC))

                def qk_proj(w_sb, dst, pi):
                    # dc-outer: one LDWEIGHTS per (w, dc) across 4 i-blocks
                    qp = [psqk.tile([128, 512], F32, tag="qkp", name=f"qp{i}")
                          for i in range(NIB)]
                    for dc in range(NDC):
                        for ib in range(NIB):
                            nc.tensor.matmul(
                                qp[ib][:],
                                w_sb[:, dc, pi * 128:(pi + 1) * 128],
                                xt[:, dc, ib * 512:(ib + 1) * 512],
                                start=(dc == 0), stop=(dc == NDC - 1))
                    for ib in range(NIB):
                        nc.vector.tensor_copy(
                            dst[pi][:, ib * 512:(ib + 1) * 512], qp[ib][:])

                # pair-0 attention (ib=0) needs v tiles 0..3 + q/k pair 0;
                # emit those first so phase B starts as early as possible.
                for ti in range(4):
                    v_proj(ti)
                qk_proj(wq_sb, qt, 0)
                qk_proj(wk_sb, kt, 0)
                for ti in range(4, NTT):
                    v_proj(ti)
                qk_proj(wq_sb, qt, 1)
                qk_proj(wk_sb, kt, 1)

            # ---------------- phase B: causal attention ----------------
            with tc.tile_pool(name="phBpt", bufs=3) as pbpt, \
                 tc.tile_pool(name="phBn", bufs=2) as pbn, \
                 tc.tile_pool(name="psBst", bufs=2, space="PSUM") as psbst, \
                 tc.tile_pool(name="psBy", bufs=2, space="PSUM") as psby:
                for pi in range(2):
                    for ib in range(NIB):
                        jlast = JPB * ib + JPB - 1
                        yab = psby.tile([128, 2, 512], F32, tag="yab")
                        for jc in range(jlast + 1):
                            off = 128 * (jc - JPB * ib) if jc >= JPB * ib else 0
                            stab = psbst.tile([128, 2, 512], F32, tag="stab")
                            ptab = pbpt.tile([128, 2, 512], BF16, tag="ptab")
                            js = slice(jc * 128, (jc + 1) * 128)
                            isl = slice(ib * 512 + off, (ib + 1) * 512)
                            nc.tensor.matmul(
                                stab[:, 0, off:512], kt[pi][0:64, js],
                                qt[pi][0:64, isl], start=True, stop=True)
                            nc.tensor.matmul(
                                stab[:, 1, off:512], kt[pi][64:128, js],
                                qt[pi][64:128, isl], start=True, stop=True,
                                tile_position=(64, 0))
                            nc.scalar.activation(
                                ptab[:, :, off:512], stab[:, :, off:512],
                                AF.Exp, scale=0.125)
                            if jc >= JPB * ib:  # diagonal chunk: mask triangle
                                nc.vector.tensor_mul(
                                    ptab[:, 0, off:off + 128],
                                    ptab[:, 0, off:off + 128], tri[:])
                                nc.vector.tensor_mul(
                                    ptab[:, 1, off:off + 128],
                                    ptab[:, 1, off:off + 128], tri[:])
                            for h in range(2):
                                nc.tensor.matmul(
                                    yab[0:65, h, off:512],
                                    vones[:, jc, 2 * pi + h, 0:DH + 1],
                                    ptab[:, h, off:512],
                                    start=(jc == 0), stop=(jc == jlast))
                        # renormalize: y /= denom (row 64), rec via ACT
                        ibs = slice(ib * 512, (ib + 1) * 512)
                        lnd = pbn.tile([1, 2, 512], F32, tag="lnd")
                        rec = pbn.tile([1, 2, 512], F32R, tag="rec")
                        nc.scalar.activation(lnd[:], yab[64:65, :, :], AF.Ln)
                        nc.scalar.activation(
                            rec[:], lnd[:], AF.Exp, scale=-1.0)
                        for h in range(2):
                            # bc borrows a stab slot (tag) to stay in 8 banks
                            bc = psbst.tile([64, 512], F32, tag="stab")
                            bcs = pbn.tile([64, 512], F32R, tag="bcs")
                            nc.tensor.matmul(
                                bc[:], ones1[:], rec[0:1, h, :],
                                start=True, stop=True)
                            nc.vector.tensor_copy(bcs[:], bc[:])
                            nc.vector.tensor_mul(
                                ypair[pi][64 * h:64 * h + 64, ibs],
                                yab[0:64, h, :], bcs[:])

            # ---------------- phase C: out-projection ----------------
            with tc.tile_pool(name="phC", bufs=3) as pc_, \
                 tc.tile_pool(name="psC", bufs=4, space="PSUM") as psc:
                for ti in range(NTT):
                    ost = pc_.tile([128, D], BF16, tag="ost")
                    for eh in range(2):
                        op = psc.tile([128, 512], F32, tag="op")
                        for pi in range(2):
                            nc.tensor.matmul(
                                op[:], ypair[pi][:, ti * 128:(ti + 1) * 128],
                                wo_sb[:, pi, eh * 512:(eh + 1) * 512],
                                start=(pi == 0), stop=(pi == 1))
                        nc.vector.tensor_copy(
                            ost[:, eh * 512:(eh + 1) * 512], op[:])
                    nc.sync.dma_start(OUT[ti * 128:(ti + 1) * 128, :], ost[:])

    nc.compile()
    return nc


_NC = None


def build_in_maps(x, w_qkv, w_out):
    x = np.asarray(x, np.float32)
    w_qkv = np.asarray(w_qkv, np.float32)
    w_out = np.asarray(w_out, np.float32)

    tri = np.triu(np.ones((128, 128), np.float32)).astype(
        ml_dtypes.bfloat16)                                # tri[j,i]=1 iff j<=i
    onesc = np.ones((128, NTT, HPC, 1), ml_dtypes.bfloat16)
    ones1 = np.ones((1, 64), np.float32)

    # [d, n] -> [128, d//128, n] with partition p = d % 128 ... d = c*128+p
    def dswz(w):
        return np.ascontiguousarray(
            w.reshape(NDC, 128, -1).transpose(1, 0, 2))

    in_maps = []
    for c in range(NCORES):
        b, g = divmod(c, 4)
        xts = dswz(np.ascontiguousarray(x[b].T).reshape(D, T))
        cs = slice(g * 256, (g + 1) * 256)
        in_maps.append({
            "XT": xts,
            "WQ": dswz(np.ascontiguousarray(w_qkv[:, 0:1024][:, cs])),
            "WK": dswz(np.ascontiguousarray(w_qkv[:, 1024:2048][:, cs])),
            "WV": dswz(np.ascontiguousarray(w_qkv[:, 2048:3072][:, cs])),
            "WO": np.ascontiguousarray(
                w_out[g * 256:(g + 1) * 256, :].reshape(2, 128, D)
                .transpose(1, 0, 2)),
            "TRI": tri, "ONESC": onesc, "ONES1": ones1,
        })
    return in_maps


def kernel(x, w_qkv, w_out):
    global _NC
    if _NC is None:
        _NC = _build()

    in_maps = build_in_maps(x, w_qkv, w_out)
    res = run_bass_kernel_spmd(_NC, in_maps, core_ids=list(range(NCORES)))
    outs = [res.results[c]["OUT"].astype(np.float32) for c in range(NCORES)]
    y = np.stack([outs[0] + outs[1] + outs[2] + outs[3],
                  outs[4] + outs[5] + outs[6] + outs[7]], axis=0)
    return y.astype(np.float32)
